# revision 32
# baseline (speedup 1.0000x reference)
"""Multi-head attention on 8 TRN2 NeuronCores.

Sharding: core c -> batch b = c//2, head-group g = c%2 (8 of 16 heads).
Each core computes, for its (batch, 8 heads):
    Q^T/K^T projections (head-dim on partitions), V natural layout,
    transposed scores S^T[t,s] per head, exp on ACT, unnormalized AV^T with
    a ones-column in V producing the softmax denominator row, normalization
    via a K=1 broadcast matmul + fast reciprocal, and the partial output
    projection against this head-group's 512 rows of Wo.
Host side: inputs are pre-transposed/cast/packed per core (bf16), the two
head-group partials per batch are summed and bo added (fp32).

Row masking (scores rows s >= len zeroed pre-softmax) is folded in for
free: masked columns of x_Q^T are zeroed on the host and the Q bias is
injected via a K=1 matmul against the mask row, so masked queries get
Q[s]=0 -> uniform softmax rows, exactly matching the reference.

The emission order software-pipelines the (in-order) PE stream so matmuls
never sit behind a wait for ACT's exp: Q/K pair projections and V are
interleaved with scores of earlier heads, and uav(h) is emitted two head
slots after scores(h).
"""

import sys

sys.path.insert(0, "/opt/trn_rl_repo")

import numpy as np
import ml_dtypes

B, S, D, H, DH = 4, 1024, 1024, 16, 64
P = 128
NPAIR = 4  # head pairs per core (8 heads)
SCALE = 1.0 / 8.0  # 1/sqrt(DH), folded into Wq/bq on host

_CACHED = None


def _build():
    import concourse.bass as bass
    import concourse.mybir as mybir
    from concourse.tile import TileContext

    bf16 = mybir.dt.bfloat16
    f32 = mybir.dt.float32
    Exp = mybir.ActivationFunctionType.Exp

    nc = bass.Bass()
    xq = nc.dram_tensor("xq", [D, S], bf16, kind="ExternalInput")  # x_Q[b].T, masked cols zeroed
    xk = nc.dram_tensor("xk", [D, S], bf16, kind="ExternalInput")
    xv = nc.dram_tensor("xv", [D, S], bf16, kind="ExternalInput")
    wq = nc.dram_tensor("wq", [D, 512], bf16, kind="ExternalInput")  # pre-scaled
    wk = nc.dram_tensor("wk", [D, 512], bf16, kind="ExternalInput")
    wv = nc.dram_tensor("wv", [D, 512], bf16, kind="ExternalInput")
    wo = nc.dram_tensor("wo", [512, D], bf16, kind="ExternalInput")
    bq = nc.dram_tensor("bq", [1, 512], bf16, kind="ExternalInput")  # pre-scaled
    bk = nc.dram_tensor("bk", [1, 512], bf16, kind="ExternalInput")
    bv = nc.dram_tensor("bv", [1, 512], bf16, kind="ExternalInput")
    mask = nc.dram_tensor("mask", [1, S], bf16, kind="ExternalInput")
    out = nc.dram_tensor("out", [S, D], f32, kind="ExternalOutput")

    with TileContext(nc) as tc:
        with (
            tc.tile_pool(name="persist", bufs=1) as persist,
            tc.tile_pool(name="expp", bufs=2) as expp,
            tc.tile_pool(name="small", bufs=4) as small,
            tc.tile_pool(name="outp", bufs=2) as outp,
            tc.tile_pool(name="stagep", bufs=8) as stagep,
            tc.tile_pool(name="ps", bufs=4, space="PSUM") as psp,
            tc.tile_pool(name="ps2", bufs=2, space="PSUM") as psp2,
        ):
            def ps_tile():
                return psp.tile([P, 512], f32, tag="ps", name="ps")

            def sc_tile():
                return psp2.tile([P, 1024], f32, tag="sc", name="sc")

            # ---- constants and small rows first ----
            bq_sb = persist.tile([1, 512], bf16, tag="bq")
            bk_sb = persist.tile([1, 512], bf16, tag="bk")
            bv_sb = persist.tile([1, 512], bf16, tag="bv")
            mask_sb = persist.tile([1, S], bf16, tag="mask")
            nc.sync.dma_start(bq_sb[:], bq[:])
            nc.sync.dma_start(bk_sb[:], bk[:])
            nc.sync.dma_start(bv_sb[:], bv[:])
            nc.sync.dma_start(mask_sb[:], mask[:])
            ones_sb = persist.tile([1, 512], bf16, tag="ones")
            nc.vector.memset(ones_sb[:], 1.0)

            # weight/x tiles; DMA chunked by d-chunk so matmuls start early
            xq_sb = persist.tile([P, 8, S], bf16, tag="xq")
            xk_sb = persist.tile([P, 8, S], bf16, tag="xk")
            xv_sb = persist.tile([P, 8, S], bf16, tag="xv")
            wq_sb = persist.tile([P, 8, 512], bf16, tag="wq")
            wk_sb = persist.tile([P, 8, 512], bf16, tag="wk")
            wv_sb = persist.tile([P, 8, 512], bf16, tag="wv")
            xq_r = xq.rearrange("(c p) s -> p c s", p=P)
            xk_r = xk.rearrange("(c p) s -> p c s", p=P)
            xv_r = xv.rearrange("(c p) s -> p c s", p=P)
            wq_r = wq.rearrange("(c p) m -> p c m", p=P)
            wk_r = wk.rearrange("(c p) m -> p c m", p=P)
            wv_r = wv.rearrange("(c p) m -> p c m", p=P)
            for dc in range(8):
                nc.sync.dma_start(wq_sb[:, dc, :], wq_r[:, dc, :])
                nc.sync.dma_start(wk_sb[:, dc, :], wk_r[:, dc, :])
                nc.sync.dma_start(xq_sb[:, dc, :], xq_r[:, dc, :])
                nc.sync.dma_start(xk_sb[:, dc, :], xk_r[:, dc, :])
            for dc in range(8):
                nc.sync.dma_start(wv_sb[:, dc, :], wv_r[:, dc, :])
                nc.sync.dma_start(xv_sb[:, dc, :], xv_r[:, dc, :])

            QT = [persist.tile([P, S], bf16, tag=f"qt{p}", name=f"qt{p}") for p in range(NPAIR)]
            KT = [persist.tile([P, S], bf16, tag=f"kt{p}", name=f"kt{p}") for p in range(NPAIR)]
            AVT = [persist.tile([P, S], bf16, tag=f"avt{p}", name=f"avt{p}") for p in range(NPAIR)]
            vaug = persist.tile([P, 8, 8 * 65], bf16, tag="vaug")
            nc.vector.memset(vaug[:], 1.0)

            expS = [None] * 8
            # denominator rows, repacked [s] -> [128 partitions, 8/partition] so
            # the reciprocal runs partition-parallel, then flattened back to a
            # [1, 8, S] row tile for the K=1 broadcast matmuls.
            stages = [None] * 8
            packed = persist.tile([P, 8, 8], f32, tag="packed")
            packed_b = persist.tile([P, 8, 8], bf16, tag="packedb")
            rdrow = persist.tile([1, 8, S], bf16, tag="rdrow")

            def emit_qk_pair(p):
                for w_sb, x_sb, b_sb, dstT, masked in (
                    (wq_sb, xq_sb, bq_sb, QT, True),
                    (wk_sb, xk_sb, bk_sb, KT, False),
                ):
                    for st in range(2):
                        ps = ps_tile()
                        srhs = (
                            mask_sb[0:1, st * 512 : (st + 1) * 512]
                            if masked
                            else ones_sb[0:1, 0:512]
                        )
                        nc.tensor.matmul(
                            ps[:],
                            lhsT=b_sb[0:1, p * P : (p + 1) * P],
                            rhs=srhs,
                            start=True,
                            stop=False,
                        )
                        for dc in range(8):
                            nc.tensor.matmul(
                                ps[:],
                                lhsT=w_sb[:, dc, p * P : (p + 1) * P],
                                rhs=x_sb[:, dc, st * 512 : (st + 1) * 512],
                                start=False,
                                stop=(dc == 7),
                            )
                        nc.vector.tensor_copy(
                            dstT[p][:, st * 512 : (st + 1) * 512], ps[:]
                        )

            def emit_v(tcn):
                ps = ps_tile()
                nc.tensor.matmul(
                    ps[:],
                    lhsT=ones_sb[0:1, 0:P],
                    rhs=bv_sb[0:1, 0:512],
                    start=True,
                    stop=False,
                )
                for dc in range(8):
                    nc.tensor.matmul(
                        ps[:],
                        lhsT=xv_sb[:, dc, tcn * P : (tcn + 1) * P],
                        rhs=wv_sb[:, dc, 0:512],
                        start=False,
                        stop=(dc == 7),
                    )
                nc.vector.tensor_copy(
                    vaug[:, tcn, :].rearrange("p (h x) -> p h x", x=65)[:, :, 0:64],
                    ps[:].rearrange("p (h v) -> p h v", v=64),
                )

            def emit_scores(h):
                p, base = h // 2, 64 * (h % 2)
                expS[h] = expp.tile([P, 8, S], bf16, tag="expS", name="expS")
                for tcn in range(8):
                    ps = sc_tile()
                    for st in range(2):
                        nc.tensor.matmul(
                            ps[:, st * 512 : (st + 1) * 512],
                            lhsT=KT[p][base : base + 64, tcn * P : (tcn + 1) * P],
                            rhs=QT[p][base : base + 64, st * 512 : (st + 1) * 512],
                            start=True,
                            stop=True,
                        )
                    nc.scalar.activation(expS[h][:, tcn, :], ps[:], Exp)

            def emit_uav(h):
                p, base = h // 2, 64 * (h % 2)
                stage = stagep.tile([1, S], f32, tag="stage", name="stage")
                stages[h] = stage
                for st in range(2):
                    psu = ps_tile()
                    for tcn in range(8):
                        nc.tensor.matmul(
                            psu[0:65, :],
                            lhsT=vaug[:, tcn, h * 65 : (h + 1) * 65],
                            rhs=expS[h][:, tcn, st * 512 : (st + 1) * 512],
                            start=(tcn == 0),
                            stop=(tcn == 7),
                        )
                    nc.vector.tensor_copy(
                        AVT[p][base : base + 64, st * 512 : (st + 1) * 512],
                        psu[0:64, :],
                    )
                    nc.vector.tensor_copy(
                        stage[0:1, st * 512 : (st + 1) * 512], psu[64:65, :]
                    )
            def emit_pack(h):
                nc.sync.dma_start(
                    packed[:, h, :],
                    stages[h][0:1, :].rearrange("o (p j) -> o p j", j=8),
                )

            def emit_recip(i):
                hs = slice(4 * i, 4 * i + 4)
                nc.vector.reciprocal(packed[:, hs, :], packed[:, hs, :])
                nc.vector.tensor_copy(packed_b[:, hs, :], packed[:, hs, :])
                for h in range(4 * i, 4 * i + 4):
                    nc.sync.dma_start(
                        rdrow[0:1, h, :].rearrange("o (p j) -> o p j", j=8),
                        packed_b[:, h, :],
                    )

            def emit_norm(h):
                # AVT[h] *= 1/denom[h,s], broadcast across the 64 v-partitions
                # via a K=1 outer-product matmul of the reciprocal row.
                p, base = h // 2, 64 * (h % 2)
                for st in range(2):
                    psr = ps_tile()
                    nc.tensor.matmul(
                        psr[0:64, :],
                        lhsT=ones_sb[0:1, 0:64],
                        rhs=rdrow[0:1, h, st * 512 : (st + 1) * 512],
                        start=True,
                        stop=True,
                    )
                    av = AVT[p][base : base + 64, st * 512 : (st + 1) * 512]
                    nc.vector.tensor_mul(av, av, psr[0:64, :])

            # ---- software-pipelined emission ----
            emit_qk_pair(0)
            emit_scores(0)
            emit_qk_pair(1)
            emit_scores(1)
            for tcn in range(8):
                emit_v(tcn)
            emit_qk_pair(2)
            emit_scores(2)
            emit_uav(0)
            emit_pack(0)
            emit_qk_pair(3)
            emit_scores(3)
            emit_uav(1)
            emit_pack(1)
            emit_scores(4)
            emit_uav(2)
            emit_pack(2)
            emit_scores(5)
            emit_uav(3)
            emit_pack(3)
            emit_recip(0)
            emit_scores(6)
            emit_uav(4)
            emit_pack(4)
            emit_scores(7)
            emit_uav(5)
            emit_pack(5)
            emit_uav(6)
            emit_pack(6)
            for h in range(4):
                emit_norm(h)
            emit_uav(7)
            emit_pack(7)
            emit_recip(1)
            for h in range(4, 8):
                emit_norm(h)

            # ---- output projection: out[s(128/chunk), m] ----
            wo_sb = expp.tile([P, 4, D], bf16, tag="expS", name="wo_sb")
            nc.sync.dma_start(wo_sb[:], wo.rearrange("(c p) m -> p c m", p=P))
            for sc in range(8):
                osb = outp.tile([P, D], f32, tag="osb", name="osb")
                for mt in range(2):
                    ps = ps_tile()
                    for p in range(NPAIR):
                        nc.tensor.matmul(
                            ps[:],
                            lhsT=AVT[p][:, sc * P : (sc + 1) * P],
                            rhs=wo_sb[:, p, mt * 512 : (mt + 1) * 512],
                            start=(p == 0),
                            stop=(p == NPAIR - 1),
                        )
                    nc.vector.tensor_copy(osb[:, mt * 512 : (mt + 1) * 512], ps[:])
                nc.sync.dma_start(out[sc * P : (sc + 1) * P, :], osb[:])

    _split_multiwait(nc)
    return nc


def _split_multiwait(nc):
    """This container's walrus rejects >1 sync wait on CTRL-class
    instructions (Tile's exit Drain carries one per outstanding proc).
    Hoist all but the last wait onto preceding same-engine NoOps."""
    import concourse.mybir as mybir

    for f in nc.m.functions:
        for bb in f.blocks:
            insts = list(bb.instructions)
            res, changed = [], False
            for inst in insts:
                si = inst.sync_info
                waits = list(si.on_wait) if si is not None else []
                if len(waits) > 1:
                    for w in waits[:-1]:
                        res.append(
                            mybir.InstNoOp(
                                name=nc.get_next_instruction_name(),
                                sync_info=mybir.SyncInfo(on_wait=[w], on_update=[]),
                                bass_nofuse=True,
                                engine=inst.engine,
                            )
                        )
                    inst.sync_info = mybir.SyncInfo(
                        on_wait=[waits[-1]], on_update=list(si.on_update)
                    )
                    changed = True
                res.append(inst)
            if changed:
                bb.instructions = res


def _shard_inputs(x_Q, x_K, x_V, src_batch_lens, Wq, bq, Wk, bk, Wv, bv, Wo, bo):
    bf = ml_dtypes.bfloat16
    f32 = np.float32
    in_maps = []
    # head-major packed weights [D, H*DH] and biases [1, H*DH]
    wq_all = (np.asarray(Wq, f32).transpose(1, 0, 2).reshape(D, H * DH) * SCALE).astype(bf)
    wk_all = np.asarray(Wk, f32).transpose(1, 0, 2).reshape(D, H * DH).astype(bf)
    wv_all = np.asarray(Wv, f32).transpose(1, 0, 2).reshape(D, H * DH).astype(bf)
    bq_all = (np.asarray(bq, f32).reshape(1, H * DH) * SCALE).astype(bf)
    bk_all = np.asarray(bk, f32).reshape(1, H * DH).astype(bf)
    bv_all = np.asarray(bv, f32).reshape(1, H * DH).astype(bf)
    wo_bf = np.asarray(Wo, f32).astype(bf)
    for c in range(8):
        b, g = c // 2, c % 2
        ln = int(src_batch_lens[b])
        m = (np.arange(S) < ln).astype(f32)
        xqT = np.ascontiguousarray(np.asarray(x_Q[b], f32).T * m[None, :]).astype(bf)
        xkT = np.ascontiguousarray(np.asarray(x_K[b], f32).T).astype(bf)
        xvT = np.ascontiguousarray(np.asarray(x_V[b], f32).T).astype(bf)
        hs = slice(g * 512, (g + 1) * 512)
        in_maps.append(
            {
                "xq": xqT,
                "xk": xkT,
                "xv": xvT,
                "wq": np.ascontiguousarray(wq_all[:, hs]),
                "wk": np.ascontiguousarray(wk_all[:, hs]),
                "wv": np.ascontiguousarray(wv_all[:, hs]),
                "wo": np.ascontiguousarray(wo_bf[hs, :]),
                "bq": np.ascontiguousarray(bq_all[:, hs]),
                "bk": np.ascontiguousarray(bk_all[:, hs]),
                "bv": np.ascontiguousarray(bv_all[:, hs]),
                "mask": m.reshape(1, S).astype(bf),
            }
        )
    return in_maps


def kernel(**inputs):
    global _CACHED
    from concourse.bass_utils import run_bass_kernel_spmd

    if _CACHED is None:
        _CACHED = _build()
    nc = _CACHED
    in_maps = _shard_inputs(**inputs)
    res = run_bass_kernel_spmd(nc, in_maps, core_ids=list(range(8)))
    bo = np.asarray(inputs["bo"], np.float32)
    out = np.empty((B, S, D), np.float32)
    for b in range(B):
        out[b] = res.results[2 * b]["out"] + res.results[2 * b + 1]["out"] + bo[None, :]
    return out


# revision 33
# speedup vs baseline: 1.0250x; 1.0250x over previous
"""Multi-head attention on 8 TRN2 NeuronCores.

Sharding: core c -> batch b = c//2, head-group g = c%2 (8 of 16 heads).
Each core computes, for its (batch, 8 heads):
    Q^T/K^T projections (head-dim on partitions), V natural layout,
    transposed scores S^T[t,s] per head, exp on ACT, unnormalized AV^T with
    a ones-column in V producing the softmax denominator row, normalization
    via a K=1 broadcast matmul + fast reciprocal, and the partial output
    projection against this head-group's 512 rows of Wo.
Host side: inputs are pre-transposed/cast/packed per core (bf16), the two
head-group partials per batch are summed and bo added (fp32).

Row masking (scores rows s >= len zeroed pre-softmax) is folded in for
free: masked columns of x_Q^T are zeroed on the host and the Q bias is
injected via a K=1 matmul against the mask row, so masked queries get
Q[s]=0 -> uniform softmax rows, exactly matching the reference.

The emission order software-pipelines the (in-order) PE stream so matmuls
never sit behind a wait for ACT's exp: Q/K pair projections and V are
interleaved with scores of earlier heads, and uav(h) is emitted two head
slots after scores(h).
"""

import sys

sys.path.insert(0, "/opt/trn_rl_repo")

import numpy as np
import ml_dtypes

B, S, D, H, DH = 4, 1024, 1024, 16, 64
P = 128
NPAIR = 4  # head pairs per core (8 heads)
SCALE = 1.0 / 8.0  # 1/sqrt(DH), folded into Wq/bq on host

_CACHED = None


def _build():
    import concourse.bass as bass
    import concourse.mybir as mybir
    from concourse.tile import TileContext

    bf16 = mybir.dt.bfloat16
    f32 = mybir.dt.float32
    Exp = mybir.ActivationFunctionType.Exp

    nc = bass.Bass()
    xq = nc.dram_tensor("xq", [D, S], bf16, kind="ExternalInput")  # x_Q[b].T, masked cols zeroed
    xk = nc.dram_tensor("xk", [D, S], bf16, kind="ExternalInput")
    xv = nc.dram_tensor("xv", [D, S], bf16, kind="ExternalInput")
    wq = nc.dram_tensor("wq", [D, 512], bf16, kind="ExternalInput")  # pre-scaled
    wk = nc.dram_tensor("wk", [D, 512], bf16, kind="ExternalInput")
    wv = nc.dram_tensor("wv", [D, 512], bf16, kind="ExternalInput")
    wo = nc.dram_tensor("wo", [512, D], bf16, kind="ExternalInput")
    bq = nc.dram_tensor("bq", [1, 512], bf16, kind="ExternalInput")  # pre-scaled
    bk = nc.dram_tensor("bk", [1, 512], bf16, kind="ExternalInput")
    bv = nc.dram_tensor("bv", [1, 512], bf16, kind="ExternalInput")
    mask = nc.dram_tensor("mask", [1, S], bf16, kind="ExternalInput")
    out = nc.dram_tensor("out", [S, D], f32, kind="ExternalOutput")

    with TileContext(nc) as tc:
        with (
            tc.tile_pool(name="persist", bufs=1) as persist,
            tc.tile_pool(name="expp", bufs=2) as expp,
            tc.tile_pool(name="small", bufs=4) as small,
            tc.tile_pool(name="outp", bufs=2) as outp,
            tc.tile_pool(name="stagep", bufs=8) as stagep,
            tc.tile_pool(name="ps", bufs=4, space="PSUM") as psp,
            tc.tile_pool(name="ps2", bufs=2, space="PSUM") as psp2,
        ):
            def ps_tile():
                return psp.tile([P, 512], f32, tag="ps", name="ps")

            def sc_tile():
                return psp2.tile([P, 1024], f32, tag="sc", name="sc")

            # ---- constants and small rows first ----
            bq_sb = persist.tile([1, 512], bf16, tag="bq")
            bk_sb = persist.tile([1, 512], bf16, tag="bk")
            bv_sb = persist.tile([1, 512], bf16, tag="bv")
            mask_sb = persist.tile([1, S], bf16, tag="mask")
            nc.sync.dma_start(bq_sb[:], bq[:])
            nc.sync.dma_start(bk_sb[:], bk[:])
            nc.sync.dma_start(bv_sb[:], bv[:])
            nc.sync.dma_start(mask_sb[:], mask[:])
            ones_sb = persist.tile([1, 512], bf16, tag="ones")
            nc.vector.memset(ones_sb[:], 1.0)

            # weight/x tiles; DMA chunked by d-chunk so matmuls start early
            xq_sb = persist.tile([P, 8, S], bf16, tag="xq")
            xk_sb = persist.tile([P, 8, S], bf16, tag="xk")
            xv_sb = persist.tile([P, 8, S], bf16, tag="xv")
            wq_sb = persist.tile([P, 8, 512], bf16, tag="wq")
            wk_sb = persist.tile([P, 8, 512], bf16, tag="wk")
            wv_sb = persist.tile([P, 8, 512], bf16, tag="wv")
            xq_r = xq.rearrange("(c p) s -> p c s", p=P)
            xk_r = xk.rearrange("(c p) s -> p c s", p=P)
            xv_r = xv.rearrange("(c p) s -> p c s", p=P)
            wq_r = wq.rearrange("(c p) m -> p c m", p=P)
            wk_r = wk.rearrange("(c p) m -> p c m", p=P)
            wv_r = wv.rearrange("(c p) m -> p c m", p=P)
            for dc in range(8):
                nc.sync.dma_start(wq_sb[:, dc, :], wq_r[:, dc, :])
                nc.sync.dma_start(wk_sb[:, dc, :], wk_r[:, dc, :])
                nc.sync.dma_start(xq_sb[:, dc, :], xq_r[:, dc, :])
                nc.sync.dma_start(xk_sb[:, dc, :], xk_r[:, dc, :])
            for dc in range(8):
                nc.sync.dma_start(wv_sb[:, dc, :], wv_r[:, dc, :])
                nc.sync.dma_start(xv_sb[:, dc, :], xv_r[:, dc, :])

            QT = [persist.tile([P, S], bf16, tag=f"qt{p}", name=f"qt{p}") for p in range(NPAIR)]
            KT = [persist.tile([P, S], bf16, tag=f"kt{p}", name=f"kt{p}") for p in range(NPAIR)]
            AVT = [persist.tile([P, S], bf16, tag=f"avt{p}", name=f"avt{p}") for p in range(NPAIR)]
            vaug = persist.tile([P, 8, 8 * 65], bf16, tag="vaug")
            nc.vector.memset(vaug[:], 1.0)

            expS = [None] * 8
            # denominator rows, repacked [s] -> [128 partitions, 8/partition] so
            # the reciprocal runs partition-parallel, then flattened back to a
            # [1, 8, S] row tile for the K=1 broadcast matmuls.
            stages = [None] * 8
            packed = persist.tile([P, 8, 8], f32, tag="packed")
            packed_b = persist.tile([P, 8, 8], bf16, tag="packedb")
            rdrow = persist.tile([1, 8, S], bf16, tag="rdrow")

            def emit_qk_pair(p):
                for w_sb, x_sb, b_sb, dstT, masked in (
                    (wq_sb, xq_sb, bq_sb, QT, True),
                    (wk_sb, xk_sb, bk_sb, KT, False),
                ):
                    for st in range(2):
                        ps = ps_tile()
                        srhs = (
                            mask_sb[0:1, st * 512 : (st + 1) * 512]
                            if masked
                            else ones_sb[0:1, 0:512]
                        )
                        nc.tensor.matmul(
                            ps[:],
                            lhsT=b_sb[0:1, p * P : (p + 1) * P],
                            rhs=srhs,
                            start=True,
                            stop=False,
                        )
                        for dc in range(8):
                            nc.tensor.matmul(
                                ps[:],
                                lhsT=w_sb[:, dc, p * P : (p + 1) * P],
                                rhs=x_sb[:, dc, st * 512 : (st + 1) * 512],
                                start=False,
                                stop=(dc == 7),
                            )
                        nc.vector.tensor_copy(
                            dstT[p][:, st * 512 : (st + 1) * 512], ps[:]
                        )

            def emit_v(tcn):
                ps = ps_tile()
                nc.tensor.matmul(
                    ps[:],
                    lhsT=ones_sb[0:1, 0:P],
                    rhs=bv_sb[0:1, 0:512],
                    start=True,
                    stop=False,
                )
                for dc in range(8):
                    nc.tensor.matmul(
                        ps[:],
                        lhsT=xv_sb[:, dc, tcn * P : (tcn + 1) * P],
                        rhs=wv_sb[:, dc, 0:512],
                        start=False,
                        stop=(dc == 7),
                    )
                nc.vector.tensor_copy(
                    vaug[:, tcn, :].rearrange("p (h x) -> p h x", x=65)[:, :, 0:64],
                    ps[:].rearrange("p (h v) -> p h v", v=64),
                )

            def emit_scores(h):
                p, base = h // 2, 64 * (h % 2)
                expS[h] = expp.tile([P, 8, S], bf16, tag="expS", name="expS")
                for tcn in range(8):
                    ps = sc_tile()
                    for st in range(2):
                        nc.tensor.matmul(
                            ps[:, st * 512 : (st + 1) * 512],
                            lhsT=KT[p][base : base + 64, tcn * P : (tcn + 1) * P],
                            rhs=QT[p][base : base + 64, st * 512 : (st + 1) * 512],
                            start=True,
                            stop=True,
                        )
                    nc.scalar.activation(expS[h][:, tcn, :], ps[:], Exp)

            def emit_uav(h):
                p, base = h // 2, 64 * (h % 2)
                stage = stagep.tile([1, S], f32, tag="stage", name="stage")
                stages[h] = stage
                for st in range(2):
                    psu = ps_tile()
                    for tcn in range(8):
                        nc.tensor.matmul(
                            psu[0:65, :],
                            lhsT=vaug[:, tcn, h * 65 : (h + 1) * 65],
                            rhs=expS[h][:, tcn, st * 512 : (st + 1) * 512],
                            start=(tcn == 0),
                            stop=(tcn == 7),
                        )
                    nc.vector.tensor_copy(
                        AVT[p][base : base + 64, st * 512 : (st + 1) * 512],
                        psu[0:64, :],
                    )
                    nc.vector.tensor_copy(
                        stage[0:1, st * 512 : (st + 1) * 512], psu[64:65, :]
                    )
            def emit_pack(h):
                nc.sync.dma_start(
                    packed[:, h, :],
                    stages[h][0:1, :].rearrange("o (p j) -> o p j", j=8),
                )

            def emit_recip(i):
                hs = slice(4 * i, 4 * i + 4)
                nc.vector.reciprocal(packed[:, hs, :], packed[:, hs, :])
                nc.vector.tensor_copy(packed_b[:, hs, :], packed[:, hs, :])
                for h in range(4 * i, 4 * i + 4):
                    nc.sync.dma_start(
                        rdrow[0:1, h, :].rearrange("o (p j) -> o p j", j=8),
                        packed_b[:, h, :],
                    )

            def emit_norm(h):
                # AVT[h] *= 1/denom[h,s], broadcast across the 64 v-partitions
                # via a K=1 outer-product matmul of the reciprocal row.
                p, base = h // 2, 64 * (h % 2)
                for st in range(2):
                    psr = ps_tile()
                    nc.tensor.matmul(
                        psr[0:64, :],
                        lhsT=ones_sb[0:1, 0:64],
                        rhs=rdrow[0:1, h, st * 512 : (st + 1) * 512],
                        start=True,
                        stop=True,
                    )
                    av = AVT[p][base : base + 64, st * 512 : (st + 1) * 512]
                    nc.vector.tensor_mul(av, av, psr[0:64, :])

            # ---- software-pipelined emission ----
            emit_qk_pair(0)
            emit_scores(0)
            emit_qk_pair(1)
            emit_scores(1)
            for tcn in range(8):
                emit_v(tcn)
            emit_qk_pair(2)
            emit_scores(2)
            emit_uav(0)
            emit_pack(0)
            emit_qk_pair(3)
            emit_scores(3)
            emit_uav(1)
            emit_pack(1)
            emit_scores(4)
            emit_uav(2)
            emit_pack(2)
            emit_scores(5)
            emit_uav(3)
            emit_pack(3)
            emit_recip(0)
            emit_scores(6)
            emit_uav(4)
            emit_pack(4)
            emit_scores(7)
            emit_uav(5)
            emit_pack(5)
            emit_uav(6)
            emit_pack(6)
            emit_uav(7)
            emit_pack(7)
            emit_recip(1)
            for h in range(8):
                emit_norm(h)

            # ---- output projection: out[s(128/chunk), m] ----
            wo_sb = expp.tile([P, 4, D], bf16, tag="expS", name="wo_sb")
            nc.sync.dma_start(wo_sb[:], wo.rearrange("(c p) m -> p c m", p=P))
            for sc in range(8):
                osb = outp.tile([P, D], f32, tag="osb", name="osb")
                for mt in range(2):
                    ps = ps_tile()
                    for p in range(NPAIR):
                        nc.tensor.matmul(
                            ps[:],
                            lhsT=AVT[p][:, sc * P : (sc + 1) * P],
                            rhs=wo_sb[:, p, mt * 512 : (mt + 1) * 512],
                            start=(p == 0),
                            stop=(p == NPAIR - 1),
                        )
                    nc.vector.tensor_copy(osb[:, mt * 512 : (mt + 1) * 512], ps[:])
                nc.sync.dma_start(out[sc * P : (sc + 1) * P, :], osb[:])

    _split_multiwait(nc)
    return nc


def _split_multiwait(nc):
    """This container's walrus rejects >1 sync wait on CTRL-class
    instructions (Tile's exit Drain carries one per outstanding proc).
    Hoist all but the last wait onto preceding same-engine NoOps."""
    import concourse.mybir as mybir

    for f in nc.m.functions:
        for bb in f.blocks:
            insts = list(bb.instructions)
            res, changed = [], False
            for inst in insts:
                si = inst.sync_info
                waits = list(si.on_wait) if si is not None else []
                if len(waits) > 1:
                    for w in waits[:-1]:
                        res.append(
                            mybir.InstNoOp(
                                name=nc.get_next_instruction_name(),
                                sync_info=mybir.SyncInfo(on_wait=[w], on_update=[]),
                                bass_nofuse=True,
                                engine=inst.engine,
                            )
                        )
                    inst.sync_info = mybir.SyncInfo(
                        on_wait=[waits[-1]], on_update=list(si.on_update)
                    )
                    changed = True
                res.append(inst)
            if changed:
                bb.instructions = res


def _shard_inputs(x_Q, x_K, x_V, src_batch_lens, Wq, bq, Wk, bk, Wv, bv, Wo, bo):
    bf = ml_dtypes.bfloat16
    f32 = np.float32
    in_maps = []
    # head-major packed weights [D, H*DH] and biases [1, H*DH]
    wq_all = (np.asarray(Wq, f32).transpose(1, 0, 2).reshape(D, H * DH) * SCALE).astype(bf)
    wk_all = np.asarray(Wk, f32).transpose(1, 0, 2).reshape(D, H * DH).astype(bf)
    wv_all = np.asarray(Wv, f32).transpose(1, 0, 2).reshape(D, H * DH).astype(bf)
    bq_all = (np.asarray(bq, f32).reshape(1, H * DH) * SCALE).astype(bf)
    bk_all = np.asarray(bk, f32).reshape(1, H * DH).astype(bf)
    bv_all = np.asarray(bv, f32).reshape(1, H * DH).astype(bf)
    wo_bf = np.asarray(Wo, f32).astype(bf)
    for c in range(8):
        b, g = c // 2, c % 2
        ln = int(src_batch_lens[b])
        m = (np.arange(S) < ln).astype(f32)
        xqT = np.ascontiguousarray(np.asarray(x_Q[b], f32).T * m[None, :]).astype(bf)
        xkT = np.ascontiguousarray(np.asarray(x_K[b], f32).T).astype(bf)
        xvT = np.ascontiguousarray(np.asarray(x_V[b], f32).T).astype(bf)
        hs = slice(g * 512, (g + 1) * 512)
        in_maps.append(
            {
                "xq": xqT,
                "xk": xkT,
                "xv": xvT,
                "wq": np.ascontiguousarray(wq_all[:, hs]),
                "wk": np.ascontiguousarray(wk_all[:, hs]),
                "wv": np.ascontiguousarray(wv_all[:, hs]),
                "wo": np.ascontiguousarray(wo_bf[hs, :]),
                "bq": np.ascontiguousarray(bq_all[:, hs]),
                "bk": np.ascontiguousarray(bk_all[:, hs]),
                "bv": np.ascontiguousarray(bv_all[:, hs]),
                "mask": m.reshape(1, S).astype(bf),
            }
        )
    return in_maps


def kernel(**inputs):
    global _CACHED
    from concourse.bass_utils import run_bass_kernel_spmd

    if _CACHED is None:
        _CACHED = _build()
    nc = _CACHED
    in_maps = _shard_inputs(**inputs)
    res = run_bass_kernel_spmd(nc, in_maps, core_ids=list(range(8)))
    bo = np.asarray(inputs["bo"], np.float32)
    out = np.empty((B, S, D), np.float32)
    for b in range(B):
        out[b] = res.results[2 * b]["out"] + res.results[2 * b + 1]["out"] + bo[None, :]
    return out


# revision 37
# speedup vs baseline: 1.0520x; 1.0263x over previous
"""Multi-head attention on 8 TRN2 NeuronCores.

Sharding: core c -> batch b = c//2, head-group g = c%2 (8 of 16 heads).
Each core computes, for its (batch, 8 heads):
    Q^T/K^T projections (head-dim on partitions), V natural layout,
    transposed scores S^T[t,s] per head, exp on ACT, unnormalized AV^T with
    a ones-column in V producing the softmax denominator row, normalization
    via a K=1 broadcast matmul + fast reciprocal, and the partial output
    projection against this head-group's 512 rows of Wo.
Host side: inputs are pre-transposed/cast/packed per core (bf16), the two
head-group partials per batch are summed and bo added (fp32).

Row masking (scores rows s >= len zeroed pre-softmax) is folded in for
free: masked columns of x_Q^T are zeroed on the host and the Q bias is
injected via a K=1 matmul against the mask row, so masked queries get
Q[s]=0 -> uniform softmax rows, exactly matching the reference.

The emission order software-pipelines the (in-order) PE stream so matmuls
never sit behind a wait for ACT's exp: Q/K pair projections and V are
interleaved with scores of earlier heads, and uav(h) is emitted two head
slots after scores(h).
"""

import sys

sys.path.insert(0, "/opt/trn_rl_repo")

import numpy as np
import ml_dtypes

B, S, D, H, DH = 4, 1024, 1024, 16, 64
P = 128
NPAIR = 4  # head pairs per core (8 heads)
SCALE = 1.0 / 8.0  # 1/sqrt(DH), folded into Wq/bq on host

_CACHED = None


def _build():
    import concourse.bass as bass
    import concourse.mybir as mybir
    from concourse.tile import TileContext

    bf16 = mybir.dt.bfloat16
    f32 = mybir.dt.float32
    Exp = mybir.ActivationFunctionType.Exp

    nc = bass.Bass()
    xq = nc.dram_tensor("xq", [D, S], bf16, kind="ExternalInput")  # x_Q[b].T, masked cols zeroed
    xk = nc.dram_tensor("xk", [D, S], bf16, kind="ExternalInput")
    xv = nc.dram_tensor("xv", [D, S], bf16, kind="ExternalInput")
    wq = nc.dram_tensor("wq", [D, 512], bf16, kind="ExternalInput")  # pre-scaled
    wk = nc.dram_tensor("wk", [D, 512], bf16, kind="ExternalInput")
    wv = nc.dram_tensor("wv", [D, 512], bf16, kind="ExternalInput")
    wo = nc.dram_tensor("wo", [512, D], bf16, kind="ExternalInput")
    bqc = nc.dram_tensor("bq", [1, 512], f32, kind="ExternalInput")  # pre-scaled
    bkc = nc.dram_tensor("bk", [1, 512], f32, kind="ExternalInput")
    bv = nc.dram_tensor("bv", [1, 512], bf16, kind="ExternalInput")
    mask = nc.dram_tensor("mask", [1, S], bf16, kind="ExternalInput")
    out = nc.dram_tensor("out", [S, D], f32, kind="ExternalOutput")

    with TileContext(nc) as tc:
        with (
            tc.tile_pool(name="persist", bufs=1) as persist,
            tc.tile_pool(name="expp", bufs=2) as expp,
            tc.tile_pool(name="small", bufs=4) as small,
            tc.tile_pool(name="outp", bufs=2) as outp,
            tc.tile_pool(name="stagep", bufs=8) as stagep,
            tc.tile_pool(name="ps", bufs=4, space="PSUM") as psp,
            tc.tile_pool(name="ps2", bufs=2, space="PSUM") as psp2,
        ):
            def ps_tile():
                return psp.tile([P, 512], f32, tag="ps", name="ps")

            def sc_tile():
                return psp2.tile([P, 1024], f32, tag="sc", name="sc")

            # ---- constants and small rows first ----
            bv_sb = persist.tile([1, 512], bf16, tag="bv")
            mask_sb = persist.tile([1, S], bf16, tag="mask")
            nc.sync.dma_start(bv_sb[:], bv[:])
            nc.sync.dma_start(mask_sb[:], mask[:])
            ones_sb = persist.tile([1, 512], bf16, tag="ones")
            nc.vector.memset(ones_sb[:], 1.0)
            bqc_sb = persist.tile([P, 4], f32, tag="bqc")
            bkc_sb = persist.tile([P, 4], f32, tag="bkc")
            nc.sync.dma_start(
                bqc_sb[:], bqc.rearrange("o (c p) -> p c o", p=P)[:, :, 0]
            )
            nc.sync.dma_start(
                bkc_sb[:], bkc.rearrange("o (c p) -> p c o", p=P)[:, :, 0]
            )
            mask_bc = persist.tile([P, S], bf16, tag="mask_bc")

            # weight/x tiles; DMA chunked by d-chunk so matmuls start early
            xq_sb = persist.tile([P, 8, S], bf16, tag="xq")
            xk_sb = persist.tile([P, 8, S], bf16, tag="xk")
            xv_sb = persist.tile([P, 8, S], bf16, tag="xv")
            wq_sb = persist.tile([P, 8, 512], bf16, tag="wq")
            wk_sb = persist.tile([P, 8, 512], bf16, tag="wk")
            wv_sb = persist.tile([P, 8, 512], bf16, tag="wv")
            xq_r = xq.rearrange("(c p) s -> p c s", p=P)
            xk_r = xk.rearrange("(c p) s -> p c s", p=P)
            xv_r = xv.rearrange("(c p) s -> p c s", p=P)
            wq_r = wq.rearrange("(c p) m -> p c m", p=P)
            wk_r = wk.rearrange("(c p) m -> p c m", p=P)
            wv_r = wv.rearrange("(c p) m -> p c m", p=P)
            for dc in range(8):
                nc.sync.dma_start(wq_sb[:, dc, :], wq_r[:, dc, :])
                nc.sync.dma_start(wk_sb[:, dc, :], wk_r[:, dc, :])
                nc.sync.dma_start(xq_sb[:, dc, :], xq_r[:, dc, :])
                nc.sync.dma_start(xk_sb[:, dc, :], xk_r[:, dc, :])
            for dc in range(8):
                nc.sync.dma_start(wv_sb[:, dc, :], wv_r[:, dc, :])
                nc.sync.dma_start(xv_sb[:, dc, :], xv_r[:, dc, :])

            QT = [persist.tile([P, S], bf16, tag=f"qt{p}", name=f"qt{p}") for p in range(NPAIR)]
            KT = [persist.tile([P, S], bf16, tag=f"kt{p}", name=f"kt{p}") for p in range(NPAIR)]
            AVT = [persist.tile([P, S], bf16, tag=f"avt{p}", name=f"avt{p}") for p in range(NPAIR)]
            vaug = persist.tile([P, 8, 8 * 65], bf16, tag="vaug")
            nc.vector.memset(vaug[:], 1.0)

            expS = [None] * 8
            # denominator rows, repacked [s] -> [128 partitions, 8/partition] so
            # the reciprocal runs partition-parallel, then flattened back to a
            # [1, 8, S] row tile for the K=1 broadcast matmuls.
            stages = [None] * 8
            packed = persist.tile([P, 8, 8], f32, tag="packed")
            packed_b = persist.tile([P, 8, 8], bf16, tag="packedb")
            rdrow = persist.tile([1, 8, S], bf16, tag="rdrow")

            def emit_qk_pair(p):
                for w_sb, x_sb, b_col, dstT, masked in (
                    (wq_sb, xq_sb, bqc_sb, QT, True),
                    (wk_sb, xk_sb, bkc_sb, KT, False),
                ):
                    for st in range(2):
                        ps = ps_tile()
                        for dc in range(8):
                            nc.tensor.matmul(
                                ps[:],
                                lhsT=w_sb[:, dc, p * P : (p + 1) * P],
                                rhs=x_sb[:, dc, st * 512 : (st + 1) * 512],
                                start=(dc == 0),
                                stop=(dc == 7),
                            )
                        dst = dstT[p][:, st * 512 : (st + 1) * 512]
                        if masked:
                            nc.vector.scalar_tensor_tensor(
                                dst,
                                ps[:],
                                b_col[:, p : p + 1],
                                mask_bc[:, st * 512 : (st + 1) * 512],
                                mybir.AluOpType.add,
                                mybir.AluOpType.mult,
                            )
                        else:
                            nc.vector.tensor_scalar_add(
                                dst, ps[:], b_col[:, p : p + 1]
                            )

            def emit_v(tcn):
                ps = ps_tile()
                nc.tensor.matmul(
                    ps[:],
                    lhsT=ones_sb[0:1, 0:P],
                    rhs=bv_sb[0:1, 0:512],
                    start=True,
                    stop=False,
                )
                for dc in range(8):
                    nc.tensor.matmul(
                        ps[:],
                        lhsT=xv_sb[:, dc, tcn * P : (tcn + 1) * P],
                        rhs=wv_sb[:, dc, 0:512],
                        start=False,
                        stop=(dc == 7),
                    )
                nc.vector.tensor_copy(
                    vaug[:, tcn, :].rearrange("p (h x) -> p h x", x=65)[:, :, 0:64],
                    ps[:].rearrange("p (h v) -> p h v", v=64),
                )

            def emit_scores(h):
                p, base = h // 2, 64 * (h % 2)
                expS[h] = expp.tile([P, 8, S], bf16, tag="expS", name="expS")
                for tcn in range(8):
                    ps = sc_tile()
                    for st in range(2):
                        nc.tensor.matmul(
                            ps[:, st * 512 : (st + 1) * 512],
                            lhsT=KT[p][base : base + 64, tcn * P : (tcn + 1) * P],
                            rhs=QT[p][base : base + 64, st * 512 : (st + 1) * 512],
                            start=True,
                            stop=True,
                        )
                    nc.scalar.activation(expS[h][:, tcn, :], ps[:], Exp)

            def emit_uav(h):
                p, base = h // 2, 64 * (h % 2)
                stage = stagep.tile([1, S], f32, tag="stage", name="stage")
                stages[h] = stage
                for st in range(2):
                    psu = ps_tile()
                    for tcn in range(8):
                        nc.tensor.matmul(
                            psu[0:65, :],
                            lhsT=vaug[:, tcn, h * 65 : (h + 1) * 65],
                            rhs=expS[h][:, tcn, st * 512 : (st + 1) * 512],
                            start=(tcn == 0),
                            stop=(tcn == 7),
                        )
                    nc.vector.tensor_copy(
                        AVT[p][base : base + 64, st * 512 : (st + 1) * 512],
                        psu[0:64, :],
                    )
                    nc.vector.tensor_copy(
                        stage[0:1, st * 512 : (st + 1) * 512], psu[64:65, :]
                    )
            def emit_pack(h):
                nc.sync.dma_start(
                    packed[:, h, :],
                    stages[h][0:1, :].rearrange("o (p j) -> o p j", j=8),
                )

            def emit_recip(i):
                hs = slice(4 * i, 4 * i + 4)
                nc.vector.reciprocal(packed[:, hs, :], packed[:, hs, :])
                nc.vector.tensor_copy(packed_b[:, hs, :], packed[:, hs, :])
                for h in range(4 * i, 4 * i + 4):
                    nc.sync.dma_start(
                        rdrow[0:1, h, :].rearrange("o (p j) -> o p j", j=8),
                        packed_b[:, h, :],
                    )

            def emit_norm(h):
                # AVT[h] *= 1/denom[h,s], broadcast across the 64 v-partitions
                # via a K=1 outer-product matmul of the reciprocal row.
                p, base = h // 2, 64 * (h % 2)
                for st in range(2):
                    psr = ps_tile()
                    nc.tensor.matmul(
                        psr[0:64, :],
                        lhsT=ones_sb[0:1, 0:64],
                        rhs=rdrow[0:1, h, st * 512 : (st + 1) * 512],
                        start=True,
                        stop=True,
                    )
                    av = AVT[p][base : base + 64, st * 512 : (st + 1) * 512]
                    nc.vector.tensor_mul(av, av, psr[0:64, :])

            # mask broadcast [128, S] for the fused Q bias+mask epilogue
            for st in range(2):
                psm = ps_tile()
                nc.tensor.matmul(
                    psm[:],
                    lhsT=ones_sb[0:1, 0:P],
                    rhs=mask_sb[0:1, st * 512 : (st + 1) * 512],
                    start=True,
                    stop=True,
                )
                nc.vector.tensor_copy(mask_bc[:, st * 512 : (st + 1) * 512], psm[:])

            # ---- software-pipelined emission ----
            emit_qk_pair(0)
            emit_scores(0)
            emit_qk_pair(1)
            emit_scores(1)
            for tcn in range(8):
                emit_v(tcn)
            emit_qk_pair(2)
            emit_scores(2)
            emit_uav(0)
            emit_pack(0)
            emit_qk_pair(3)
            emit_scores(3)
            emit_uav(1)
            emit_pack(1)
            emit_scores(4)
            emit_uav(2)
            emit_pack(2)
            emit_scores(5)
            emit_uav(3)
            emit_pack(3)
            emit_recip(0)
            emit_scores(6)
            emit_uav(4)
            emit_pack(4)
            emit_scores(7)
            emit_uav(5)
            emit_pack(5)
            emit_uav(6)
            emit_pack(6)
            emit_uav(7)
            emit_pack(7)
            emit_recip(1)
            for h in range(8):
                emit_norm(h)

            # ---- output projection: out[s(128/chunk), m] ----
            wo_sb = expp.tile([P, 4, D], bf16, tag="expS", name="wo_sb")
            nc.sync.dma_start(wo_sb[:], wo.rearrange("(c p) m -> p c m", p=P))
            for sc in range(8):
                osb = outp.tile([P, D], f32, tag="osb", name="osb")
                for mt in range(2):
                    ps = ps_tile()
                    for p in range(NPAIR):
                        nc.tensor.matmul(
                            ps[:],
                            lhsT=AVT[p][:, sc * P : (sc + 1) * P],
                            rhs=wo_sb[:, p, mt * 512 : (mt + 1) * 512],
                            start=(p == 0),
                            stop=(p == NPAIR - 1),
                        )
                    nc.vector.tensor_copy(osb[:, mt * 512 : (mt + 1) * 512], ps[:])
                nc.sync.dma_start(out[sc * P : (sc + 1) * P, :], osb[:])

    _split_multiwait(nc)
    return nc


def _split_multiwait(nc):
    """This container's walrus rejects >1 sync wait on CTRL-class
    instructions (Tile's exit Drain carries one per outstanding proc).
    Hoist all but the last wait onto preceding same-engine NoOps."""
    import concourse.mybir as mybir

    for f in nc.m.functions:
        for bb in f.blocks:
            insts = list(bb.instructions)
            res, changed = [], False
            for inst in insts:
                si = inst.sync_info
                waits = list(si.on_wait) if si is not None else []
                if len(waits) > 1:
                    for w in waits[:-1]:
                        res.append(
                            mybir.InstNoOp(
                                name=nc.get_next_instruction_name(),
                                sync_info=mybir.SyncInfo(on_wait=[w], on_update=[]),
                                bass_nofuse=True,
                                engine=inst.engine,
                            )
                        )
                    inst.sync_info = mybir.SyncInfo(
                        on_wait=[waits[-1]], on_update=list(si.on_update)
                    )
                    changed = True
                res.append(inst)
            if changed:
                bb.instructions = res


def _shard_inputs(x_Q, x_K, x_V, src_batch_lens, Wq, bq, Wk, bk, Wv, bv, Wo, bo):
    bf = ml_dtypes.bfloat16
    f32 = np.float32
    in_maps = []
    # head-major packed weights [D, H*DH] and biases [1, H*DH]
    wq_all = (np.asarray(Wq, f32).transpose(1, 0, 2).reshape(D, H * DH) * SCALE).astype(bf)
    wk_all = np.asarray(Wk, f32).transpose(1, 0, 2).reshape(D, H * DH).astype(bf)
    wv_all = np.asarray(Wv, f32).transpose(1, 0, 2).reshape(D, H * DH).astype(bf)
    bq_all = (np.asarray(bq, f32).reshape(1, H * DH) * SCALE).astype(f32)
    bk_all = np.asarray(bk, f32).reshape(1, H * DH).astype(f32)
    bv_all = np.asarray(bv, f32).reshape(1, H * DH).astype(bf)
    wo_bf = np.asarray(Wo, f32).astype(bf)
    for c in range(8):
        b, g = c // 2, c % 2
        ln = int(src_batch_lens[b])
        m = (np.arange(S) < ln).astype(f32)
        xqT = np.ascontiguousarray(np.asarray(x_Q[b], f32).T * m[None, :]).astype(bf)
        xkT = np.ascontiguousarray(np.asarray(x_K[b], f32).T).astype(bf)
        xvT = np.ascontiguousarray(np.asarray(x_V[b], f32).T).astype(bf)
        hs = slice(g * 512, (g + 1) * 512)
        in_maps.append(
            {
                "xq": xqT,
                "xk": xkT,
                "xv": xvT,
                "wq": np.ascontiguousarray(wq_all[:, hs]),
                "wk": np.ascontiguousarray(wk_all[:, hs]),
                "wv": np.ascontiguousarray(wv_all[:, hs]),
                "wo": np.ascontiguousarray(wo_bf[hs, :]),
                "bq": np.ascontiguousarray(bq_all[:, hs]),
                "bk": np.ascontiguousarray(bk_all[:, hs]),
                "bv": np.ascontiguousarray(bv_all[:, hs]),
                "mask": m.reshape(1, S).astype(bf),
            }
        )
    return in_maps


def kernel(**inputs):
    global _CACHED
    from concourse.bass_utils import run_bass_kernel_spmd

    if _CACHED is None:
        _CACHED = _build()
    nc = _CACHED
    in_maps = _shard_inputs(**inputs)
    res = run_bass_kernel_spmd(nc, in_maps, core_ids=list(range(8)))
    bo = np.asarray(inputs["bo"], np.float32)
    out = np.empty((B, S, D), np.float32)
    for b in range(B):
        out[b] = res.results[2 * b]["out"] + res.results[2 * b + 1]["out"] + bo[None, :]
    return out
